# revision 17
# baseline (speedup 1.0000x reference)
"""Causal multi-head self-attention (RoPE) on 8 TRN2 NeuronCores.

Sharding: core c = (batch b = c//2, head-group g = c%2). Each core computes
QKV projections for its 8 heads on its batch, RoPE, causal attention in
transposed-score space (scores^T = [k_part, q_free]; softmax sums via a
ones-column appended to V), a partial out-projection over its 512 head dims,
then a pairwise AllReduce [[0,1],[2,3],[4,5],[6,7]] sums the two head-group
partials into the full output (bf16 on the wire; host widens to fp32).

Schedule: q-chunks in order (0,3,2,1). scn0's attention interleaves with the
V projection; each later scn interleaves the previous chunk's out-projection
+ AllReduce into its pr loop so the PE never drains and every collective
hides behind attention compute. Mask-muls and softmax-normalize muls run on
GpSimd (the DVE is otherwise a co-bottleneck); softmax 1/sum runs per-scn as
one [4,1024] ln+exp pair on the scalar engine.

Shapes (hardcoded): x [4, 2048, 1024], Wq/Wk/Wv/Wo [1024, 1024],
token_positions [2048]. D_K=64, N_HEADS=16, THETA=10000.
"""
import copy
import sys

sys.path.insert(0, "/opt/trn_rl_repo")

import ml_dtypes
import numpy as np

import bass_rust
import concourse.bass as bass
import concourse.mybir as mybir
import concourse.tile as tile
from concourse.bass_utils import run_bass_kernel_spmd

P = 128
S = 2048
D = 1024
OG = 512          # head dims per group (8 heads x 64)
DK = 64
THETA = 10000.0
F32 = mybir.dt.float32
BF16 = mybir.dt.bfloat16
F8 = mybir.dt.float8e4
BF = ml_dtypes.bfloat16
RSCALE = 32.0     # fp8 range scale folded into the cos/sin tables

_cache = {}


def _split_multi_waits(nc, max_waits=1):
    """The staged walrus build rejects instructions carrying more than one
    attached sem-wait ("Too many sync wait commands"). Hoist excess waits
    into standalone single-wait EventSemaphore instructions just before the
    offending instruction (same engine, so semantics are identical)."""
    n_split = 0
    new_module = copy.replace(nc.m, functions=[])
    for function in nc.m.functions:
        new_function = copy.replace(function, blocks=[])
        new_function.set_allocations_from_list(function.allocations)
        for block in function.blocks:
            new_insts = []
            for inst in block.instructions:
                si = inst.sync_info
                if si is not None and len(si.on_wait) > max_waits:
                    waits = list(si.on_wait)
                    for j, w in enumerate(waits[:-max_waits]):
                        ev = bass_rust.InstEventSemaphore(
                            name=f"{inst.name}-wsplit{j}", ins=[], outs=[]
                        )
                        ev.engine = inst.engine
                        ev.sync_info = bass_rust.SyncInfo(on_wait=[w], on_update=[])
                        new_insts.append(ev)
                        n_split += 1
                    si.on_wait = waits[-max_waits:]
                new_insts.append(inst)
            new_block = copy.replace(block, instructions=new_insts)
            new_function.blocks.append(new_block)
        new_module.functions.append(new_function)
    nc.m = new_module
    return n_split


def _build_nc(split_waits=True):
    nc = bass.Bass(num_devices=8)

    xt_e = nc.declare_dram_parameter("xt", [D, S], BF16, isOutput=False)
    wq_e = nc.declare_dram_parameter("wqt", [D, OG], BF16, isOutput=False)
    wk_e = nc.declare_dram_parameter("wkt", [D, OG], BF16, isOutput=False)
    wv_e = nc.declare_dram_parameter("wvt", [D, OG], BF16, isOutput=False)
    wo_e = nc.declare_dram_parameter("wot", [OG, D], BF16, isOutput=False)
    cos_e = nc.declare_dram_parameter("cosT", [P, S], BF16, isOutput=False)
    sin_e = nc.declare_dram_parameter("sinT", [P, S], BF16, isOutput=False)
    mb_e = nc.declare_dram_parameter("mb", [P, 4, 512], BF16, isOutput=False)
    y_ext = nc.declare_dram_parameter("y", [S, D], BF16, isOutput=True)

    ctx = tile.TileContext(nc)
    with ctx as tc, tc.tile_pool(name="persist", bufs=1) as persist, \
         tc.tile_pool(name="small", bufs=1) as small, \
         tc.tile_pool(name="wo_pool", bufs=1) as wo_pool, \
         tc.tile_pool(name="ppool", bufs=6) as ppool, \
         tc.tile_pool(name="opool", bufs=2) as opool, \
         tc.tile_pool(name="stg", bufs=2) as stgp, \
         tc.tile_pool(name="yout", bufs=3) as yout, \
         tc.tile_pool(name="sps", bufs=2, space="PSUM") as spsp, \
         tc.tile_pool(name="avps", bufs=2, space="PSUM") as avpsp, \
         tc.tile_pool(name="ardram", bufs=2, space="DRAM") as ardram:
        qkraw = persist.tile([P, 8, S], BF16, tag="qkraw")  # Q blocks 0-3, K 4-7
        v_ext = persist.tile([P, 16, 8, 66], BF16)
        # RoPE'd Q/K in fp8e4, scaled by RSCALE, laid out for DoubleRow
        # score matmuls: head h -> partitions 32*(h%4)..+32 of group h//4,
        # k-tile slot 0 = even dk dims, slot 1 = odd dk dims.
        rqh8q = persist.tile([P, 2, 2, S], F8, tag="rqh8q")
        rqh8k = persist.tile([P, 2, 2, S], F8, tag="rqh8k")
        mb_s = small.tile([P, 4, 512], BF16)
        wo_s = wo_pool.tile([P, 4, D], BF16)
        ones_bc = persist.tile([P, 64], BF16)
        # rbc (recip-broadcast) PSUM tiles borrow a slot from whichever
        # [P,512]-f32 PSUM pool is alive in the current phase (PSUM is
        # exactly full otherwise); set before each attention_scn call.
        rbc_src = [None]
        rbc_pool = lambda: rbc_src[0]  # noqa: E731

        def attention_scn(scn, steps=None, split_recip=False):
            """Scores + softmax + AV for q-chunk scn; returns the normalized
            o_chunk. `steps` is a list of closures each emitting a small
            (~0.5-1us) piece of independent PE work (V-projection or the
            previous chunk's out-projection); one is popped per pipeline
            iteration so the PE never drains and HAM stays warm."""
            steps = list(steps or [])

            def pop_step(flush=False):
                while steps:
                    steps.pop(0)()
                    if not flush:
                        break

            o_chunk = opool.tile([P, 4, 512], BF16, tag="ochunk")
            # per-pr softmax sums parked 32 partitions apart (engine writes
            # must start at a 32-aligned base partition); ln/exp then run on
            # all 128 partitions at once (lanes are parallel, unused rows
            # hold garbage that nothing reads)
            ssb = stgp.tile([P, 1024], F32, tag="ssb")
            tmp = stgp.tile([P, 1024], F32, tag="tmp")
            rq = stgp.tile([P, 1024], BF16, tag="rq")
            nk = 4 * (scn + 1)
            for pr in range(4):
                av0 = avpsp.tile([65, 512], F32, tag="av", name="av0")
                av1 = avpsp.tile([65, 512], F32, tag="av", name="av1")
                av = [av0, av1]
                # lag-2 software pipeline: AV matmuls for block j-2 issue
                # right after the score matmuls for block j, so the PE never
                # stalls on the exp chain and the diag mask-muls (split
                # across DVE and gpsimd) get two iterations of slack
                pts = {}
                for j in range(nk + 2):
                    if j < nk:
                        kb = j
                        sps = spsp.tile([P, 2, 512], F32, tag="sps")
                        for hh in range(2):
                            h = 2 * pr + hh
                            rows = slice(32 * (h % 4), 32 * (h % 4) + 32)
                            nc.tensor.matmul(
                                sps[:, hh, :],
                                rqh8k[rows, h // 4, :, kb * P:(kb + 1) * P],
                                rqh8q[rows, h // 4, :,
                                      scn * 512:(scn + 1) * 512],
                                start=True,
                                stop=True,
                                perf_mode=mybir.MatmulPerfMode.DoubleRow,
                                tile_position=(32 * (h % 4), 0),
                            )
                        pt = ppool.tile([P, 2, 512], BF16, tag="pt")
                        nc.scalar.activation(
                            pt, sps, mybir.ActivationFunctionType.Exp,
                            scale=0.125 / (RSCALE * RSCALE),
                        )
                        if kb >= 4 * scn:
                            # zero the above-diagonal probabilities (0/1 mask)
                            r = kb - 4 * scn
                            nc.vector.tensor_mul(
                                pt[:, 0, :], pt[:, 0, :], mb_s[:, r, :]
                            )
                            nc.gpsimd.tensor_mul(
                                pt[:, 1, :], pt[:, 1, :], mb_s[:, r, :]
                            )
                        pts[kb] = pt
                    if j >= 2:
                        kb = j - 2
                        pt = pts.pop(kb)
                        for hh in range(2):
                            nc.tensor.matmul(
                                av[hh],
                                v_ext[:, kb, 2 * pr + hh, 0:65],
                                pt[:, hh, :],
                                start=(kb == 0),
                                stop=(kb == nk - 1),
                            )
                    pop_step()
                # drain: o_chunk rows 0-63 = head 2pr, 64-127 = head 2pr+1;
                # softmax sums (PSUM row 64) go to ssb for the scn-batched
                # reciprocal. Cheap DVE copies so the AV banks free fast.
                nc.vector.tensor_copy(
                    ssb[32 * pr:32 * pr + 1, 0:512], av0[64:65, :]
                )
                nc.vector.tensor_copy(
                    ssb[32 * pr:32 * pr + 1, 512:1024], av1[64:65, :]
                )
                nc.vector.tensor_copy(o_chunk[0:64, pr, :], av0[0:64, :])
                nc.vector.tensor_copy(o_chunk[64:128, pr, :], av1[0:64, :])
                if split_recip and pr == 2:
                    # normalize prs 0-2 while pr3's attention runs, so only
                    # pr3's short recip chain gates the final out-projection
                    recip(ssb, o_chunk, tmp, rq, range(3), 0, 96, rbc_pool)
                pop_step()
            if split_recip:
                recip(ssb, o_chunk, tmp, rq, (3,), 96, 32, rbc_pool)
            else:
                recip(ssb, o_chunk, tmp, rq, range(4), 0, P, rbc_pool)
            pop_step(flush=True)
            return o_chunk

        def recip(ssb, o_chunk, tmp, rq, prs, p0, np_, rbc_pool):
            """softmax 1/s = exp(-ln(s)) on the scalar engine, batched over
            `prs` (per-pr sums parked 32 partitions apart; lanes parallel so
            one [*,1024] ln+exp pair covers them all). DVE reciprocal is an
            8x-iterative op and too slow. The recip row is written bf16 and
            broadcast across 64 partitions by a rank-1 PE matmul
            (ones ⊗ row -> PSUM, ~213ns) — no DRAM bounce, no DGE latency.
            bf16 denominators cost ~0.4% relative, well inside the gate."""
            pool, tag = rbc_pool()
            nc.scalar.activation(
                tmp[p0:p0 + np_, :], ssb[p0:p0 + np_, :],
                mybir.ActivationFunctionType.Ln,
            )
            nc.scalar.activation(
                rq[p0:p0 + np_, :], tmp[p0:p0 + np_, :],
                mybir.ActivationFunctionType.Exp, scale=-1.0,
            )
            for pr in prs:
                rbc = pool.tile([P, 512], F32, tag=tag, name="rbc")
                for hh in range(2):
                    nc.tensor.matmul(
                        rbc[64 * hh:64 * hh + 64, :],
                        ones_bc[32 * pr:32 * pr + 1, :],
                        rq[32 * pr:32 * pr + 1, 512 * hh:512 * hh + 512],
                        start=True,
                        stop=True,
                        tile_position=(32 * pr, 64 * hh),
                    )
                nc.vector.tensor_mul(
                    o_chunk[0:64, pr, :], o_chunk[0:64, pr, :], rbc[0:64, :]
                )
                nc.vector.tensor_mul(
                    o_chunk[64:128, pr, :], o_chunk[64:128, pr, :],
                    rbc[64:128, :],
                )

        # ---- projections: Q, K (transposed out: [dims, S]), then V ----
        with tc.tile_pool(name="xtp", bufs=1) as xtp, \
             tc.tile_pool(name="wpool", bufs=2) as wpool, \
             tc.tile_pool(name="prps", bufs=2, space="PSUM") as prps:
            xt = xtp.tile([P, 8, S], BF16)
            xt_src = xt_e.ap().rearrange("(dc p) s -> p dc s", p=P)
            first = True
            for w_ext, dst_base in [(wq_e, 0), (wk_e, 4)]:
                w_sb = wpool.tile([P, 8, OG], BF16, tag="w")
                w_src = w_ext.ap().rearrange("(dc p) o -> p dc o", p=P)
                if first:
                    # per-chunk loads so the first matmul starts ~3us in;
                    # non-critical init (masks, Wo, the softmax ones column)
                    # queues behind them
                    for dc in range(8):
                        nc.sync.dma_start(out=w_sb[:, dc, :], in_=w_src[:, dc, :])
                        nc.sync.dma_start(out=xt[:, dc, :], in_=xt_src[:, dc, :])
                    nc.sync.dma_start(out=mb_s, in_=mb_e.ap())
                    nc.sync.dma_start(
                        out=wo_s,
                        in_=wo_e.ap().rearrange("(dc p) o -> p dc o", p=P),
                    )
                    # col 64 of every (sb, h) v_ext slot must be 1.0 (softmax
                    # sum column); cols 0-63 are overwritten by V-projection
                    vcol = persist.tile([P, 1], BF16)
                    nc.vector.memset(vcol, 1.0)
                    nc.vector.memset(ones_bc, 1.0)
                    nc.vector.tensor_copy(
                        v_ext[:, :, :, 64:65],
                        vcol[:, None, None, :].to_broadcast((P, 16, 8, 1)),
                    )
                    first = False
                else:
                    nc.sync.dma_start(out=w_sb, in_=w_src)
                for ob in range(4):
                    for scn in range(4):
                        ps = prps.tile([P, 512], F32, tag="projps")
                        for dc in range(8):
                            nc.tensor.matmul(
                                ps,
                                w_sb[:, dc, ob * P:(ob + 1) * P],
                                xt[:, dc, scn * 512:(scn + 1) * 512],
                                start=(dc == 0),
                                stop=(dc == 7),
                            )
                        nc.vector.tensor_copy(
                            qkraw[:, dst_base + ob, scn * 512:(scn + 1) * 512],
                            ps,
                        )
            # ---- RoPE on Q and K (in place), V projection emitted after
            # it so the PE has matmul work (V) while the DVE rotates Q/K ----
            wv_sb = wpool.tile([P, 8, OG], BF16, tag="w")
            nc.sync.dma_start(
                out=wv_sb, in_=wv_e.ap().rearrange("(dc p) o -> p dc o", p=P)
            )
            with tc.tile_pool(name="trig", bufs=1) as trig, \
                 tc.tile_pool(name="rtmp", bufs=1) as rtmp:
                cos_s = trig.tile([P, S], BF16)
                sin_s = trig.tile([P, S], BF16)
                nc.sync.dma_start(out=cos_s, in_=cos_e.ap())
                nc.sync.dma_start(out=sin_s, in_=sin_e.ap())
                # RoPE block order (0, 4, 2, 6): pair-0/1 Q then K blocks
                # first, so the first score matmuls (heads 0-3) can start
                # while heads 4-7 are still rotating. The final sub/add
                # writes RSCALE-scaled fp8 directly into the DoubleRow
                # layout (partition rows already match: row = 32*(h%4)+k),
                # so no repack is needed at all.
                for pb in (0, 4, 2, 6):
                    e_blk = qkraw[:, pb, :]
                    o_blk = qkraw[:, pb + 1, :]
                    dst = rqh8q if pb < 4 else rqh8k
                    grp = (pb % 4) // 2
                    t0 = rtmp.tile([P, S], BF16, tag="t0")
                    t1 = rtmp.tile([P, S], BF16, tag="t1")
                    t2 = rtmp.tile([P, S], BF16, tag="t2")
                    nc.vector.tensor_mul(t0, e_blk, cos_s)
                    nc.vector.tensor_mul(t1, o_blk, sin_s)
                    nc.vector.tensor_mul(t2, e_blk, sin_s)
                    # e' = e*cos - o*sin ; o' = e*sin + o*cos
                    nc.vector.tensor_mul(o_blk, o_blk, cos_s)  # o_blk := o*cos
                    nc.vector.tensor_sub(dst[:, grp, 0, :], t0, t1)
                    nc.vector.tensor_add(dst[:, grp, 1, :], t2, o_blk)

            # V projection: natural layout [s, dims] -> v_ext[:, sb, h, 0:64].
            # sb 0-3 first (what scn0's AV needs), the rest woven into scn0's
            # pipeline in half-chunk steps so the PE has work while scn0's
            # exps run.
            def vproj_steps(sb):
                st = {}

                def a():
                    st["ps"] = prps.tile(
                        [P, 512], F32, tag="projps", name="vps"
                    )
                    for dc in range(4):
                        nc.tensor.matmul(
                            st["ps"],
                            xt[:, dc, sb * P:(sb + 1) * P],
                            wv_sb[:, dc, :],
                            start=(dc == 0),
                            stop=False,
                        )

                def b():
                    for dc in range(4, 8):
                        nc.tensor.matmul(
                            st["ps"],
                            xt[:, dc, sb * P:(sb + 1) * P],
                            wv_sb[:, dc, :],
                            start=False,
                            stop=(dc == 7),
                        )
                    nc.vector.tensor_copy(
                        v_ext[:, sb, :, 0:64],
                        st["ps"].rearrange("p (h d) -> p h d", h=8),
                    )

                return [a, b]

            for sb in range(4):
                for step in vproj_steps(sb):
                    step()

            vsteps = [s for sb in range(4, 16) for s in vproj_steps(sb)]
            rbc_src[0] = (prps, "projps")
            o_chunks = {0: attention_scn(0, steps=vsteps)}

        # ---- remaining q-chunks; each interleaves the previous chunk's
        # out-projection (+ AllReduce) into its pr loop ----
        with tc.tile_pool(name="yps", bufs=2, space="PSUM") as ypsp:

            def outproj_piece(scn, o_chunk, qb, oc, st, half):
                """Half of one out-projection accumulation chain (2 of 4 db
                matmuls); second half drains to arin. Split so each woven
                step is ~0.5us of PE work."""
                if half == 0:
                    st["yps"] = ypsp.tile([P, 512], F32, tag="yps", name="yps")
                dbs = (0, 1) if half == 0 else (2, 3)
                for db in dbs:
                    nc.tensor.matmul(
                        st["yps"],
                        o_chunk[:, db, qb * P:(qb + 1) * P],
                        wo_s[:, db, oc * 512:(oc + 1) * 512],
                        start=(db == 0),
                        stop=(db == 3),
                    )
                if half == 1:
                    yt = yout.tile([P, 512], BF16, tag="yt", name="yt")
                    nc.vector.tensor_copy(yt, st["yps"])
                    nc.sync.dma_start(
                        out=_arin[scn][qb * P:(qb + 1) * P,
                                       oc * 512:(oc + 1) * 512],
                        in_=yt,
                    )

            def outproj_chunk(scn, o_chunk, qb):
                for oc in range(2):
                    st = {}
                    outproj_piece(scn, o_chunk, qb, oc, st, 0)
                    outproj_piece(scn, o_chunk, qb, oc, st, 1)

            def outproj_steps(scn, o_chunk):
                steps = []
                for qb in range(4):
                    for oc in range(2):
                        st = {}
                        for half in range(2):
                            steps.append(
                                lambda qb=qb, oc=oc, st=st, half=half:
                                outproj_piece(scn, o_chunk, qb, oc, st, half)
                            )
                steps.append(lambda: allreduce_rows(scn, 0, 4))
                return steps

            def allreduce_rows(scn, q0, q1):
                # y-write goes on the sync queue, NOT gpsimd: a gpsimd
                # dma_start would wait on AR completion at the head of the
                # gpsimd queue, stalling the next chunk's mask-muls (and so
                # the PE) for the whole collective (~15us).
                arout = ardram.tile([512, D], BF16, tag="arout", name="arout")
                nc.gpsimd.collective_compute(
                    "AllReduce",
                    mybir.AluOpType.add,
                    replica_groups=[[0, 1], [2, 3], [4, 5], [6, 7]],
                    ins=[_arin[scn][q0 * P:q1 * P, :].opt()],
                    outs=[arout[q0 * P:q1 * P, :].opt()],
                )
                nc.sync.dma_start(
                    out=y_ext.ap()[scn * 512 + q0 * P:scn * 512 + q1 * P, :],
                    in_=arout[q0 * P:q1 * P, :],
                )

            _arin = {}
            rbc_src[0] = (ypsp, "yps")

            for scn, prev in ((3, 0), (2, 3), (1, 2)):
                _arin[prev] = ardram.tile(
                    [512, D], BF16, tag="arin", name="arin"
                )
                o_chunks[scn] = attention_scn(
                    scn,
                    steps=outproj_steps(prev, o_chunks[prev]),
                    split_recip=(scn == 1),
                )

            # final chunk (scn1): nothing left to hide it behind. One
            # 512-row AllReduce beats four 128-row pieces: each collective
            # carries ~8-10us of fixed overhead (mesh events + sem waits),
            # so pipelining in 128-row pieces quadruples that overhead for
            # at most ~7us of overlapped out-proj compute.
            _arin[1] = ardram.tile([512, D], BF16, tag="arin", name="arin")
            for qb in range(4):
                outproj_chunk(1, o_chunks[1], qb)
            allreduce_rows(1, 0, 4)

    if split_waits:
        _split_multi_waits(nc)
    return nc


def _host_inputs(x, Wq, Wk, Wv, Wo, token_positions):
    """Per-core input dicts. Host work is layout-only (transpose/slice/tables)."""
    half = DK // 2
    k = np.arange(1, half + 1, dtype=np.float64)
    inv_freq = THETA ** (-(2.0 * k - 2.0) / DK)  # [32]
    pos = np.asarray(token_positions).astype(np.float64)  # [S]
    ang = pos[None, :] * inv_freq[:, None]  # [32, S]
    cosT = np.ascontiguousarray(np.tile(np.cos(ang) * RSCALE, (4, 1))).astype(BF)
    sinT = np.ascontiguousarray(np.tile(np.sin(ang) * RSCALE, (4, 1))).astype(BF)

    # permuted Q/K dim order within a group: blk(4) x h4(4) x k(32);
    # blk 0/1 = heads 0-3 evens/odds, blk 2/3 = heads 4-7 evens/odds
    perm = np.empty(OG, dtype=np.int64)
    i = 0
    for blk in range(4):
        for h4 in range(4):
            for kk in range(32):
                perm[i] = 64 * (4 * (blk // 2) + h4) + 2 * kk + (blk % 2)
                i += 1

    pp = np.arange(P)[:, None]
    ff = np.arange(512)[None, :]
    mb = np.empty((P, 4, 512), dtype=np.float32)
    for r in range(4):
        mb[:, r, :] = np.where(ff >= 128 * r + pp, 1.0, 0.0)
    mb = mb.astype(BF)

    in_maps = []
    for c in range(8):
        b = c // 2
        g = c % 2
        gd = slice(g * OG, (g + 1) * OG)  # group's head dims among 1024
        gdim = np.arange(g * OG, (g + 1) * OG)
        xt = np.ascontiguousarray(x[b].T).astype(BF)  # [D, S]
        wqt = np.ascontiguousarray(Wq[gdim[perm], :].T).astype(BF)
        wkt = np.ascontiguousarray(Wk[gdim[perm], :].T).astype(BF)
        wvt = np.ascontiguousarray(Wv[gd, :].T).astype(BF)
        wot = np.ascontiguousarray(Wo[:, gd].T).astype(BF)
        in_maps.append({
            "xt": xt, "wqt": wqt, "wkt": wkt, "wvt": wvt, "wot": wot,
            "cosT": cosT, "sinT": sinT, "mb": mb,
        })
    return in_maps


def kernel(x, Wq, Wk, Wv, Wo, token_positions, _trace=False):
    x = np.asarray(x)
    Wq, Wk, Wv, Wo = (np.asarray(w) for w in (Wq, Wk, Wv, Wo))
    token_positions = np.asarray(token_positions)
    if "nc" not in _cache:
        _cache["nc"] = _build_nc()
    nc = _cache["nc"]
    in_maps = _host_inputs(x, Wq, Wk, Wv, Wo, token_positions)
    try:
        res = run_bass_kernel_spmd(
            nc, in_maps, core_ids=list(range(8)), trace=_trace
        )
    except Exception:
        # one retry for transient runtime failures (wedged core, NRT timeout)
        res = run_bass_kernel_spmd(
            nc, in_maps, core_ids=list(range(8)), trace=_trace
        )
    _cache["last_result"] = res
    out = np.empty((x.shape[0], S, D), dtype=np.float32)
    for b in range(x.shape[0]):
        out[b] = res.results[2 * b]["y"].astype(np.float32)
    return out

